# revision 1
# baseline (speedup 1.0000x reference)
"""Trainium2 Bass kernel for cached multi-head self-attention decode step.

Problem (hardcoded):
  B=16, T=8, C=1024, n_head=16, head_dim=64, Lcache=4096, layer index 1.
  reference:
    q = x@Wq.T + bq ; key = x@Wk.T ; value = x@Wv.T + bv
    K = concat(kv_cache[:,1,0], key) ; V = concat(kv_cache[:,1,1], value)
    out = softmax((q*s)(K*s)^T) @ V @ Wo.T + bo      (s = hd**-0.25)
    returns (out, key, value)

Sharding: data-parallel over batch. 8 cores x 2 batches each. No collectives.

Per-core device algorithm:
  - q computed directly TRANSPOSED: qT[co,:] = sum_ci (Wq~ chunk).T @ xT chunk
    (Wq~ host-prescaled by 1/sqrt(hd)), ACT-copied (bias folded, fp8 out)
    straight into the block-diagonal Qbig lhsT. No PE transposes of q.
  - k/v projections natural [m, c] (needed for the key/value outputs);
    kT (for the 8 new keys) via PE transpose.
  - scores per batch: S[(h,m), l] over 8 512-wide KT windows, 8 c-chunk
    PSUM-accumulated N=512 matmuls each, Qbig stationary.
    exp fused in PSUM->SBUF ACT (logit shift ESHIFT), row-sums via accum_out,
    normalization deferred to the attention output.
  - S@V per batch: W-chunks PE-transposed (bf16) with the transposes
    software-pipelined PIPE chunks ahead of the consuming matmuls; fp8
    conversion in the PSUM->SBUF DVE copy; V streamed natural, 2 N=512
    PSUM-accumulated matmuls per l-chunk over 33 chunks (last = new kv rows).
  - out = (gathered wvT chunks) @ WoT (preloaded) + bo.

Batch-sequential phasing so scores(b1) overlaps S@V(b0). KT/V/Wq/Wkv/Wo are
host-repacked so every DMA is a contiguous [128 x >=2KB] transfer. Weights are
bf16; KV cache, Qbig and S@V weights are fp8-e4m3 (combined-metric error
~2.4e-3 vs the 2e-2 gate).
"""

import sys
import types

import numpy as np
import ml_dtypes

# ---- hardcoded problem geometry ----
B, T, C = 16, 8, 1024
H, HD = 16, 64
L = 4096            # cached length
LT = L + T          # total keys
NCORES = 8
BPC = B // NCORES   # batches per core = 2
M = BPC * T         # queries per core = 16
P = 128
CH = C // P         # 8 c-chunks
NW = L // 512       # 8 score windows of 512
NV = 4              # V l-chunks per DMA
PIPE = 3            # S@V transpose software-pipeline depth (chunks)
SCALE = float(HD) ** -0.5  # folded into Wq/bq on host

ATTN_DT = "fp8"
# softmax logit shift: exp(s + ESHIFT); cancels in normalization, keeps the
# fp8 S@V weights well inside e4m3 range.
ESHIFT = -2.0

_CACHE = {}


def _ensure_ntff_hook():
    """run_bass_kernel_spmd(trace=True) under axon needs antenv.axon_hooks;
    shim it from the boot module if the image's antenv lacks it."""
    try:
        import antenv.axon_hooks  # noqa: F401
        return
    except ImportError:
        pass
    try:
        import trn_agent_boot.trn_boot as tb
        hook = tb._ntff_profile_via_ctypes("/opt/axon/libaxon_pjrt.so")
    except Exception:
        hook = None
    mod = types.ModuleType("antenv.axon_hooks")
    mod.get_axon_ntff_profile_hook = lambda: hook
    mod.set_axon_ntff_profile_hook = lambda h: None
    sys.modules["antenv.axon_hooks"] = mod


def _build(attn_dt_name: str):
    import concourse.bacc as bacc
    import concourse.mybir as mybir
    import concourse.tile as tile
    from concourse.masks import make_identity

    f32 = mybir.dt.float32
    bf16 = mybir.dt.bfloat16
    adt = {"fp8": mybir.dt.float8e4, "bf16": bf16}[attn_dt_name]
    wdt = bf16  # PE-transpose dtype for score weights (fp8 transpose is
    #             constrained); fp8 conversion happens in the PSUM->SBUF copy.

    nc = bacc.Bacc(None, target_bir_lowering=False)

    # ---- dram I/O (all host-repacked for contiguous loads) ----
    xTr = nc.dram_tensor("xTr", [P, CH, M], bf16, kind="ExternalInput")
    KT = nc.dram_tensor("KT", [BPC, NW, P, CH * 512], adt, kind="ExternalInput")
    Vd = nc.dram_tensor("Vd", [BPC, L // (P * NV), P, NV * C], adt,
                        kind="ExternalInput")
    WqT = nc.dram_tensor("WqT", [CH, P, C], bf16, kind="ExternalInput")
    Wkv = nc.dram_tensor("Wkv", [CH, P, 2 * C], bf16, kind="ExternalInput")
    WoTd = nc.dram_tensor("WoTd", [P, CH * C], bf16, kind="ExternalInput")
    bqs = nc.dram_tensor("bqs", [P, CH], f32, kind="ExternalInput")
    bvb = nc.dram_tensor("bvb", [M, C], f32, kind="ExternalInput")
    bob = nc.dram_tensor("bob", [M, C], f32, kind="ExternalInput")
    out_d = nc.dram_tensor("out", [M, C], f32, kind="ExternalOutput")
    key_d = nc.dram_tensor("key", [M, C], f32, kind="ExternalOutput")
    val_d = nc.dram_tensor("value", [M, C], f32, kind="ExternalOutput")

    AF = mybir.ActivationFunctionType
    AX = mybir.AxisListType
    OP = mybir.AluOpType

    nt = L // P  # 32 cached l-chunks per batch

    with tile.TileContext(nc) as tc:
        with (
            tc.tile_pool(name="const", bufs=1) as cpool,
            tc.tile_pool(name="wq", bufs=8) as wqpool,
            tc.tile_pool(name="wkv", bufs=3) as wkvpool,
            tc.tile_pool(name="kt", bufs=16) as ktpool,
            tc.tile_pool(name="v", bufs=10) as vpool,
            tc.tile_pool(name="wchunk", bufs=8) as wtpool,
            tc.tile_pool(name="big", bufs=1) as big,
            tc.tile_pool(name="ps", bufs=1, space="PSUM") as pp,
        ):
            # ---- constants / preloads ----
            ident = cpool.tile([P, P], f32, tag="ident", name="ident")
            make_identity(nc, ident)
            ident_w = cpool.tile([P, P], wdt, tag="ident_w", name="ident_w")
            nc.vector.tensor_copy(out=ident_w[:], in_=ident[:])
            xT_sb = cpool.tile([P, CH, M], bf16, tag="xT", name="xT")
            nc.sync.dma_start(xT_sb[:], xTr[:])
            bqs_sb = cpool.tile([P, CH], f32, tag="bqs", name="bqs")
            nc.sync.dma_start(bqs_sb[:], bqs[:])
            eshift = cpool.tile([P, 1], f32, tag="eshift", name="eshift")
            nc.gpsimd.memset(eshift[:], ESHIFT)
            bvb_sb = cpool.tile([M, C], f32, tag="bvb", name="bvb")
            nc.sync.dma_start(bvb_sb[:], bvb[:])
            bob_sb = cpool.tile([M, C], f32, tag="bob", name="bob")
            nc.sync.dma_start(bob_sb[:], bob[:])
            wo_all = cpool.tile([P, CH, C], bf16, tag="wo", name="wo")

            # ---- qT directly from transposed projection ----
            # qT chunk co = sum_ci WqT[ci].T @ xT[ci].  One start/stop group
            # per PSUM bank (start=True clears has_written for the WHOLE bank,
            # so groups may not interleave within a bank); ping-pong t0/t1.
            wq_t = []
            for ci in range(CH):
                wq = wqpool.tile([P, C], bf16, tag="wq", name="wq")
                nc.scalar.dma_start(wq[:], WqT[ci])
                wq_t.append(wq)
            Qb = {}
            for b in range(BPC):
                Qb[b] = big.tile([P, CH, P], adt, tag=f"Qbig{b}",
                                 name=f"Qbig{b}")
                nc.gpsimd.memset(Qb[b][:], 0.0)
            for co in range(CH):
                qps = pp.tile([P, M], f32, tag=f"t{co % 2}", name=f"qps{co}")
                for ci in range(CH):
                    nc.tensor.matmul(
                        qps[:], wq_t[ci][:, co * P:(co + 1) * P],
                        xT_sb[:, ci, :],
                        start=(ci == 0), stop=(ci == CH - 1),
                    )
                # scatter into block-diagonal Qbig (bias folded, fp8 out)
                for b in range(BPC):
                    for j in range(2):
                        rows = slice(64 * j, 64 * (j + 1))
                        nc.scalar.activation(
                            Qb[b][rows, co, 16 * co + 8 * j:16 * co + 8 * j + 8],
                            qps[rows, b * T:b * T + T],
                            AF.Identity, bias=bqs_sb[rows, co:co + 1],
                        )

            # ---- per-batch state ----
            W_s, sums, rsum, wt32, ops_b, On = {}, {}, {}, {}, {}, {}
            for b in range(BPC):
                W_s[b] = big.tile([P, LT], wdt, tag=f"W{b}", name=f"W{b}")
                sums[b] = big.tile([P, NW + 1], f32, tag=f"sums{b}",
                                   name=f"sums{b}")

            kt_pre = {}

            def scores_windows(b, pre=None):
                for lw in range(NW):
                    if pre is not None:
                        kt = pre.pop(lw)
                    else:
                        kt = ktpool.tile([P, CH, 512], adt, tag="kt", name="kt")
                        nc.sync.dma_start(kt[:], KT[b, lw])
                    sp = pp.tile([P, 512], f32, tag=f"s{lw % 2}", name="sp")
                    for ci in range(CH):
                        nc.tensor.matmul(
                            sp[:], Qb[b][:, ci, :], kt[:, ci, :],
                            start=(ci == 0), stop=(ci == CH - 1),
                        )
                    nc.scalar.activation(
                        W_s[b][:, lw * 512:(lw + 1) * 512], sp[:], AF.Exp,
                        bias=eshift[:, 0:1],
                        accum_out=sums[b][:, lw:lw + 1],
                    )

            def scores_newkey(b):
                # scores against the T new keys + normalization + padded
                # transposed new-key weights (last S@V chunk lhsT)
                spn = pp.tile([P, 512], f32, tag=f"s{NW % 2}", name="spn")
                for ci in range(CH):
                    nc.tensor.matmul(
                        spn[:, 0:T], Qb[b][:, ci, :], kT[:, ci, b * T:(b + 1) * T],
                        start=(ci == 0), stop=(ci == CH - 1),
                    )
                nc.scalar.activation(
                    W_s[b][:, L:LT], spn[:, 0:T], AF.Exp,
                    bias=eshift[:, 0:1], accum_out=sums[b][:, NW:NW + 1],
                )
                rs = big.tile([P, 1], f32, tag=f"rs{b}", name=f"rs{b}")
                nc.vector.tensor_reduce(out=rs[:], in_=sums[b][:],
                                        axis=AX.X, op=OP.add)
                rsum[b] = big.tile([P, 1], f32, tag=f"rsum{b}", name=f"rsum{b}")
                nc.vector.reciprocal(rsum[b][:], rs[:])
                wn_pad = big.tile([P, M], wdt, tag=f"wn_pad{b}",
                                  name=f"wn_pad{b}")
                nc.gpsimd.memset(wn_pad[:], 0.0)
                nc.vector.tensor_copy(out=wn_pad[:, b * T:(b + 1) * T],
                                      in_=W_s[b][:, L:LT])
                tpn = pp.tile([P, P], wdt, tag=f"t{b % 2}", name="tpn")
                nc.tensor.transpose(tpn[0:M, :], wn_pad[:], ident_w)
                wt32[b] = big.tile([P, P], adt, tag=f"wt32_{b}",
                                   name=f"wt32_{b}")
                nc.gpsimd.memset(wt32[b][:], 0.0)
                nc.vector.tensor_copy(out=wt32[b][0:M, :], in_=tpn[0:M, :])

            def sv_main(b):
                ops = [pp.tile([P, 512], f32, tag=f"o{2 * b + j}",
                               name=f"sv{b}{j}") for j in range(2)]
                ops_b[b] = ops
                wts = {}
                for t in range(nt + PIPE):
                    if t < nt:
                        tpw = pp.tile([P, P], wdt, tag=f"t{t % 2}", name="tpw")
                        nc.tensor.transpose(
                            tpw[:], W_s[b][:, t * P:(t + 1) * P], ident_w)
                        wts[t] = wtpool.tile([P, P], adt, tag="wt", name="wt")
                        nc.vector.tensor_copy(out=wts[t][:], in_=tpw[:])
                    if b == 0 and t % 4 == 0 and t // 4 < NW:
                        ktp = ktpool.tile([P, CH, 512], adt, tag="kt",
                                          name="kt")
                        nc.sync.dma_start(ktp[:], KT[1, t // 4])
                        kt_pre[t // 4] = ktp
                    if t >= PIPE:
                        t_ = t - PIPE
                        if t_ % NV == 0:
                            vt = vpool.tile([P, NV, C], adt, tag="v", name="v")
                            eng = nc.sync if b == 0 else nc.scalar
                            eng.dma_start(vt[:], Vd[b, t_ // NV])
                        tt = t_ % NV
                        for j in range(2):
                            nc.tensor.matmul(
                                ops[j][:], wts[t_][:],
                                vt[:, tt, j * 512:(j + 1) * 512],
                                start=(t_ == 0), stop=False,
                            )
                        del wts[t_]

            def sv_final(b):
                ops = ops_b[b]
                for j in range(2):
                    nc.tensor.matmul(
                        ops[j][:], wt32[b][:], vpad[:, j * 512:(j + 1) * 512],
                        start=False, stop=True,
                    )
                On[b] = big.tile([P, C], wdt, tag=f"On{b}", name=f"On{b}")
                for j in range(2):
                    nc.scalar.activation(
                        On[b][:, j * 512:(j + 1) * 512], ops[j][:], AF.Copy,
                        scale=rsum[b][:],
                    )

            # wvT[c_local, (h,t)] from transposed On chunks: head of c_local<64
            # is 2ci else 2ci+1 -> two 64-partition block copies per (b, ci).
            wvT = big.tile([P, CH, M], bf16, tag="wvT", name="wvT")

            def gather(b):
                for ci in range(CH):
                    tp = pp.tile([P, P], wdt, tag=f"t{ci % 2}", name="tpg")
                    nc.tensor.transpose(
                        tp[:], On[b][:, ci * P:(ci + 1) * P], ident_w)
                    nc.vector.tensor_copy(
                        out=wvT[0:64, ci, b * T:(b + 1) * T],
                        in_=tp[0:64, 16 * ci:16 * ci + 8])
                    nc.vector.tensor_copy(
                        out=wvT[64:P, ci, b * T:(b + 1) * T],
                        in_=tp[64:P, 16 * ci + 8:16 * ci + 16])

            # batch-sequential with late-bound kv-projection: nothing in
            # the PE FIFO ahead of scores(0)/sv_main(0) depends on Wkv.
            scores_windows(0)
            sv_main(0)
            # ---- k/v natural projections ----
            ps_kv = [pp.tile([M, 512], f32, tag=t, name=f"kv{t}")
                     for t in ("o2", "o3", "s0", "s1")]
            for ci in range(CH):
                wkv = wkvpool.tile([P, 2 * C], bf16, tag="wkv", name="wkv")
                nc.scalar.dma_start(wkv[:], Wkv[ci])
                for j, ps in enumerate(ps_kv):
                    nc.tensor.matmul(
                        ps[:], xT_sb[:, ci, :], wkv[:, j * 512:(j + 1) * 512],
                        start=(ci == 0), stop=(ci == CH - 1),
                    )
            k_nat = big.tile([P, C], f32, tag="k_nat", name="k_nat")
            v_nat = big.tile([P, C], f32, tag="v_nat", name="v_nat")
            for t in (k_nat, v_nat):
                nc.gpsimd.memset(t[:], 0.0)
            for j in range(2):
                sl = slice(j * 512, (j + 1) * 512)
                nc.scalar.copy(k_nat[0:M, sl], ps_kv[j][:])
                nc.scalar.copy(v_nat[0:M, sl], ps_kv[2 + j][:])
            nc.vector.tensor_add(out=v_nat[0:M, :], in0=v_nat[0:M, :],
                                 in1=bvb_sb[:])
            nc.scalar.dma_start(key_d[:], k_nat[0:M, :])
            nc.scalar.dma_start(val_d[:], v_nat[0:M, :])
            # kT (new keys) via PE transpose
            kT = big.tile([P, CH, M], adt, tag="kT", name="kT")
            for ci in range(CH):
                tp = pp.tile([P, P], f32, tag=f"t{ci % 2}", name="tpk")
                nc.tensor.transpose(tp[:], k_nat[:, ci * P:(ci + 1) * P], ident)
                nc.vector.tensor_copy(out=kT[:, ci, :], in_=tp[:, 0:M])
            # padded new-v rows for the last S@V chunk (both batches)
            vpad = big.tile([P, C], adt, tag="vpad", name="vpad")
            nc.gpsimd.memset(vpad[:], 0.0)
            nc.vector.tensor_copy(out=vpad[0:M, :], in_=v_nat[0:M, :])

            scores_newkey(0)
            sv_final(0)
            # WoT rides the scalar queue between V(b0) and V(b1)
            nc.scalar.dma_start(wo_all[:], WoTd[:])
            scores_windows(1, pre=kt_pre)
            scores_newkey(1)
            gather(0)
            sv_main(1)
            sv_final(1)
            # gather(1) fused with the out-projection: outproj chunk ci only
            # needs wvT[:, ci, :], so its matmuls chase the per-ci copies.
            ps_fin = [pp.tile([M, 512], f32, tag=f"s{j}", name=f"fin{j}")
                      for j in range(2)]
            for ci in range(CH):
                tp = pp.tile([P, P], wdt, tag=f"t{ci % 2}", name="tpg")
                nc.tensor.transpose(
                    tp[:], On[1][:, ci * P:(ci + 1) * P], ident_w)
                nc.vector.tensor_copy(
                    out=wvT[0:64, ci, T:2 * T],
                    in_=tp[0:64, 16 * ci:16 * ci + 8])
                nc.vector.tensor_copy(
                    out=wvT[64:P, ci, T:2 * T],
                    in_=tp[64:P, 16 * ci + 8:16 * ci + 16])
                for j in range(2):
                    nc.tensor.matmul(
                        ps_fin[j][:], wvT[:, ci, :],
                        wo_all[:, ci, j * 512:(j + 1) * 512],
                        start=(ci == 0), stop=(ci == CH - 1),
                    )
            fin = big.tile([M, C], f32, tag="fin", name="fin")
            for j in range(2):
                sl = slice(j * 512, (j + 1) * 512)
                nc.vector.tensor_add(out=fin[:, sl], in0=ps_fin[j][:],
                                     in1=bob_sb[:, sl])
                nc.sync.dma_start(out_d[:, sl], fin[:, sl])

    nc.compile()
    return nc


def _prep_host(x, kv_cache, Wq, bq, Wk, Wv, bv, Wo, bo, attn_dt_name):
    np_adt = {"fp8": ml_dtypes.float8_e4m3, "bf16": ml_dtypes.bfloat16}[
        attn_dt_name]
    bf16 = ml_dtypes.bfloat16
    f32 = np.float32
    x = np.asarray(x, f32)
    kv = np.asarray(kv_cache)
    Wq = np.asarray(Wq, f32); bq = np.asarray(bq, f32)
    Wk = np.asarray(Wk, f32); Wv = np.asarray(Wv, f32); bv = np.asarray(bv, f32)
    Wo = np.asarray(Wo, f32); bo = np.asarray(bo, f32)

    # K-cache / V-cache repacked so every device DMA is a fully contiguous
    # [128 x 4KB] transfer:
    #   KT[b, w, p, ci*512 + j] = K[b, w*512 + j, ci*128 + p]
    #   Vd[b, s, p, tt*C + c]   = V[b, (s*NV + tt)*128 + p, c]
    KT_all = np.asarray(kv[:, 1, 0], f32).transpose(0, 2, 1).reshape(
        B, CH, P, NW, 512).transpose(0, 3, 2, 1, 4)
    KT_all = np.ascontiguousarray(KT_all).astype(np_adt)
    V_all = np.asarray(kv[:, 1, 1], f32).reshape(
        B, L // (P * NV), NV, P, C).transpose(0, 1, 3, 2, 4)
    V_all = np.ascontiguousarray(V_all).astype(np_adt)

    WqT8 = np.ascontiguousarray(Wq.T * SCALE).reshape(CH, P, C).astype(bf16)
    Wkv8 = np.ascontiguousarray(
        np.concatenate([Wk.T, Wv.T], axis=1)).reshape(CH, P, 2 * C).astype(bf16)
    WoT8 = np.ascontiguousarray(Wo.T).reshape(CH, P, C).transpose(
        1, 0, 2).reshape(P, CH * C)
    WoT8 = np.ascontiguousarray(WoT8).astype(bf16)
    bqs = np.ascontiguousarray((bq * SCALE).reshape(CH, P).T)  # [P, CH]
    bvb = np.ascontiguousarray(np.tile(bv, (M, 1)))
    bob = np.ascontiguousarray(np.tile(bo, (M, 1)))

    in_maps = []
    for c in range(NCORES):
        xc = x[c * BPC:(c + 1) * BPC].reshape(M, C)
        xTr = np.ascontiguousarray(
            xc.reshape(M, CH, P).transpose(2, 1, 0)).astype(bf16)
        in_maps.append({
            "xTr": xTr,
            "KT": np.ascontiguousarray(KT_all[c * BPC:(c + 1) * BPC]).reshape(
                BPC, NW, P, CH * 512),
            "Vd": np.ascontiguousarray(V_all[c * BPC:(c + 1) * BPC]).reshape(
                BPC, L // (P * NV), P, NV * C),
            "WqT": WqT8, "Wkv": Wkv8, "WoTd": WoT8,
            "bqs": bqs, "bvb": bvb, "bob": bob,
        })
    return in_maps


def kernel(x, kv_cache, Wq, bq, Wk, Wv, bv, Wo, bo, _trace=False, _tmpdir=None):
    from concourse.bass_utils import run_bass_kernel_spmd

    _ensure_ntff_hook()
    key = ATTN_DT
    if key not in _CACHE:
        _CACHE[key] = _build(key)
    nc = _CACHE[key]

    in_maps = _prep_host(x, kv_cache, Wq, bq, Wk, Wv, bv, Wo, bo, key)
    res = run_bass_kernel_spmd(
        nc, in_maps, core_ids=list(range(NCORES)),
        trace=_trace, tmpdir=_tmpdir,
    )
    out = np.empty((B, T, C), np.float32)
    key_o = np.empty((B, T, C), np.float32)
    val_o = np.empty((B, T, C), np.float32)
    for c in range(NCORES):
        r = res.results[c]
        sl = slice(c * BPC, (c + 1) * BPC)
        out[sl] = r["out"].reshape(BPC, T, C)
        key_o[sl] = r["key"].reshape(BPC, T, C)
        val_o[sl] = r["value"].reshape(BPC, T, C)
    kernel._last_exec_time_ns = res.exec_time_ns
    kernel._last_results = res
    return (out, key_o, val_o)



# revision 6
# speedup vs baseline: 1.5104x; 1.5104x over previous
"""Trainium2 Bass kernel for cached multi-head self-attention decode step.

Problem (hardcoded):
  B=16, T=8, C=1024, n_head=16, head_dim=64, Lcache=4096, layer index 1.
  reference:
    q = x@Wq.T + bq ; key = x@Wk.T ; value = x@Wv.T + bv
    K = concat(kv_cache[:,1,0], key) ; V = concat(kv_cache[:,1,1], value)
    out = softmax((q*s)(K*s)^T) @ V @ Wo.T + bo      (s = hd**-0.25)
    returns (out, key, value)

Sharding: data-parallel over batch. 8 cores x 2 batches each. No collectives.

v2 vs baseline (143us):
  - fp8 DoubleRow matmuls (contract 256/instr) for scores, S@V, q-proj and
    out-proj: ~2x PE throughput on the attention streams.
  - all transposes via regular matmul with identity rhs (lhsT = data): ~85ns
    each vs ~275ns PE transpose-mode.
  - Wq/Wo cast to fp8 on host (-2MB DMA), Wkv single 4MB bf16 DMA,
    V in 1MB tiles; DMA issue order matches consumption order so the PE
    starts ~4us in and the 16 SDMA engines stay fed.
"""

import sys
import types

import numpy as np
import ml_dtypes

# ---- hardcoded problem geometry ----
B, T, C = 16, 8, 1024
H, HD = 16, 64
L = 4096            # cached length
LT = L + T          # total keys
NCORES = 8
BPC = B // NCORES   # batches per core = 2
M = BPC * T         # queries per core = 16
P = 128
CH = C // P         # 8 c-chunks
NW = L // 512       # 8 score windows of 512
NV = 8              # V l-chunks (128 rows) per DMA tile (1MB transfers)
NDR = L // 256      # 16 DoubleRow l-pair chunks per batch
PIPE = 2            # W^T transpose software-pipeline depth (DR chunks)
SCALE = float(HD) ** -0.5  # folded into Wq/bq on host

# softmax logit shift: exp(s + ESHIFT); cancels in normalization, keeps the
# fp8 S@V weights well inside e4m3 range.
ESHIFT = -2.0

_CACHE = {}


def _ensure_ntff_hook():
    """run_bass_kernel_spmd(trace=True) under axon needs antenv.axon_hooks;
    shim it from the boot module if the image's antenv lacks it."""
    try:
        import antenv.axon_hooks  # noqa: F401
        return
    except ImportError:
        pass
    try:
        import trn_agent_boot.trn_boot as tb
        hook = tb._ntff_profile_via_ctypes("/opt/axon/libaxon_pjrt.so")
    except Exception:
        hook = None
    mod = types.ModuleType("antenv.axon_hooks")
    mod.get_axon_ntff_profile_hook = lambda: hook
    mod.set_axon_ntff_profile_hook = lambda h: None
    sys.modules["antenv.axon_hooks"] = mod


def _build():
    import concourse.bacc as bacc
    import concourse.mybir as mybir
    import concourse.tile as tile
    from concourse.masks import make_identity

    f32 = mybir.dt.float32
    bf16 = mybir.dt.bfloat16
    fp8 = mybir.dt.float8e4
    DR = mybir.MatmulPerfMode.DoubleRow

    nc = bacc.Bacc(None, target_bir_lowering=False)

    # ---- dram I/O (all host-repacked for contiguous loads) ----
    xT8 = nc.dram_tensor("xT8", [P, CH, M], fp8, kind="ExternalInput")
    xTb = nc.dram_tensor("xTb", [P, CH, M], bf16, kind="ExternalInput")
    Wq8 = nc.dram_tensor("Wq8", [P, CH * C], fp8, kind="ExternalInput")
    Wo8 = nc.dram_tensor("Wo8", [P, CH * C], fp8, kind="ExternalInput")
    Wkvd = nc.dram_tensor("Wkvd", [P, CH * 2 * C], bf16, kind="ExternalInput")
    KT = nc.dram_tensor("KT", [BPC, NW, P, CH * 512], fp8, kind="ExternalInput")
    Vd = nc.dram_tensor("Vd", [BPC, L // (P * NV), P, NV * C], fp8,
                        kind="ExternalInput")
    bqs = nc.dram_tensor("bqs", [P, CH], f32, kind="ExternalInput")
    bvb = nc.dram_tensor("bvb", [M, C], f32, kind="ExternalInput")
    bob = nc.dram_tensor("bob", [M, C], f32, kind="ExternalInput")
    out_d = nc.dram_tensor("out", [M, C], f32, kind="ExternalOutput")
    key_d = nc.dram_tensor("key", [M, C], f32, kind="ExternalOutput")
    val_d = nc.dram_tensor("value", [M, C], f32, kind="ExternalOutput")

    AF = mybir.ActivationFunctionType
    AX = mybir.AxisListType
    OP = mybir.AluOpType

    NS = L // (P * NV)  # 4 V dma tiles per batch

    with tile.TileContext(nc) as tc:
        with (
            tc.tile_pool(name="const", bufs=1) as cpool,
            tc.tile_pool(name="kt", bufs=15) as ktpool,
            tc.tile_pool(name="v", bufs=6) as vpool,
            tc.tile_pool(name="nat", bufs=2) as natpool,
            tc.tile_pool(name="wchunk", bufs=PIPE + 2) as wtpool,
            tc.tile_pool(name="big", bufs=1) as big,
            tc.tile_pool(name="ps", bufs=1, space="PSUM") as pp,
        ):
            # ---------------- DMA issue order == consumption order --------
            # sync (HWDGE) queue carries the big ordered stream; scalar
            # carries tiny consts; outputs ride scalar at the end.
            xT8_sb = cpool.tile([P, CH, M], fp8, tag="xT8", name="xT8")
            nc.sync.dma_start(xT8_sb[:], xT8[:])
            xTb_sb = cpool.tile([P, CH, M], bf16, tag="xTb", name="xTb")
            nc.sync.dma_start(xTb_sb[:], xTb[:])
            wq_sb = cpool.tile([P, CH, C], fp8, tag="wq", name="wq")
            nc.sync.dma_start(wq_sb[:], Wq8[:])
            kts = {}
            for b, w in [(0, w) for w in range(NW)]:
                kts[(b, w)] = ktpool.tile([P, CH, 512], fp8, tag="kt",
                                          name="kt")
                nc.sync.dma_start(kts[(b, w)][:], KT[b, w])
            vts = {}
            # interleave: V0 s, then 2 KT1 windows, ...
            w1 = 0
            for s in range(NS):
                vts[(0, s)] = vpool.tile([P, NV, C], fp8, tag="v", name="v")
                nc.sync.dma_start(vts[(0, s)][:], Vd[0, s])
                for _ in range(2):
                    kts[(1, w1)] = ktpool.tile([P, CH, 512], fp8, tag="kt",
                                               name="kt")
                    nc.sync.dma_start(kts[(1, w1)][:], KT[1, w1])
                    w1 += 1
            wkv_sb = cpool.tile([P, CH, 2 * C], bf16, tag="wkv", name="wkv")
            nc.sync.dma_start(wkv_sb[:], Wkvd[:])
            for s in range(NS):
                vts[(1, s)] = vpool.tile([P, NV, C], fp8, tag="v", name="v")
                nc.sync.dma_start(vts[(1, s)][:], Vd[1, s])
            wo_sb = cpool.tile([P, CH, C], fp8, tag="wo", name="wo")
            nc.sync.dma_start(wo_sb[:], Wo8[:])

            bqs_sb = cpool.tile([P, CH], f32, tag="bqs", name="bqs")
            nc.scalar.dma_start(bqs_sb[:], bqs[:])
            bvb_sb = cpool.tile([M, C], bf16, tag="bvb", name="bvb")
            nc.gpsimd.dma_start(bvb_sb[:], bvb[:])
            bob_sb = cpool.tile([M, C], bf16, tag="bob", name="bob")
            nc.gpsimd.dma_start(bob_sb[:], bob[:])

            # ---- constants ----
            ident = cpool.tile([P, P], f32, tag="ident", name="ident")
            make_identity(nc, ident)
            ident_b = cpool.tile([P, P], bf16, tag="ident_b", name="ident_b")
            nc.vector.tensor_copy(out=ident_b[:], in_=ident[:])
            ident_8 = cpool.tile([P, P], fp8, tag="ident_8", name="ident_8")
            nc.vector.tensor_copy(out=ident_8[:], in_=ident[:])
            eshift = cpool.tile([P, 1], f32, tag="eshift", name="eshift")
            nc.gpsimd.memset(eshift[:], ESHIFT)

            Qb = {}
            for b in range(BPC):
                Qb[b] = big.tile([P, CH, P], fp8, tag=f"Qbig{b}",
                                 name=f"Qbig{b}")
                nc.gpsimd.memset(Qb[b][:], 0.0)

            # ---------------- Phase A: q projection (fp8 DR) --------------
            # q_nat[m, c_out] = sum_cin x[m, cin] Wq~[c_out, cin]
            q_bf = big.tile([M, C], bf16, tag="q_bf", name="q_bf")
            for j in range(2):
                qps = pp.tile([M, 512], f32, tag=f"s{j}", name=f"qps{j}")
                for ci in range(0, CH, 2):
                    nc.tensor.matmul(
                        qps[:], xT8_sb[:, ci:ci + 2, :],
                        wq_sb[:, ci:ci + 2, j * 512:(j + 1) * 512],
                        start=(ci == 0), stop=(ci == CH - 2), perf_mode=DR,
                    )
                nc.scalar.copy(q_bf[:, j * 512:(j + 1) * 512], qps[:])
            # qT chunks via matmul-transpose; scatter block-diagonal into
            # Qbig (bias folded, fp8 out).
            for co in range(CH):
                tpq = pp.tile([P, M], f32, tag=f"t{co % 2}", name="tpq")
                nc.tensor.matmul(
                    tpq[:], q_bf[:, co * P:(co + 1) * P],
                    ident_b[0:M, 0:M], start=True, stop=True,
                )
                for b in range(BPC):
                    for j in range(2):
                        rows = slice(64 * j, 64 * (j + 1))
                        nc.scalar.activation(
                            Qb[b][rows, co, 16 * co + 8 * j:16 * co + 8 * j + 8],
                            tpq[rows, b * T:b * T + T],
                            AF.Identity, bias=bqs_sb[rows, co:co + 1],
                        )

            # ---------------- per-batch state ------------------------------
            W_s, sums, rsum, wt32, ops_b, On = {}, {}, {}, {}, {}, {}
            for b in range(BPC):
                W_s[b] = big.tile([P, LT], fp8, tag=f"W{b}", name=f"W{b}")
                sums[b] = big.tile([P, NW + 1], f32, tag=f"sums{b}",
                                   name=f"sums{b}")
                On[b] = big.tile([P, C], bf16, tag=f"On{b}", name=f"On{b}")

            def scores_windows(b):
                for lw in range(NW):
                    kt = kts[(b, lw)]
                    sp = pp.tile([P, 512], f32, tag=f"s{lw % 2}", name="sp")
                    for ci in range(0, CH, 2):
                        nc.tensor.matmul(
                            sp[:], Qb[b][:, ci:ci + 2, :], kt[:, ci:ci + 2, :],
                            start=(ci == 0), stop=(ci == CH - 2), perf_mode=DR,
                        )
                    nc.scalar.activation(
                        W_s[b][:, lw * 512:(lw + 1) * 512], sp[:], AF.Exp,
                        bias=eshift[:, 0:1],
                        accum_out=sums[b][:, lw:lw + 1],
                    )

            def sv_main(b, extra=None):
                """DR S@V over the 16 cached l-pair chunks, with the W^T
                matmul-transposes software-pipelined PIPE chunks ahead.
                extra: list of callables injected into the loop (one per
                iteration) to interleave other PE work (e.g. gather(b-1))."""
                ops = [pp.tile([P, 512], f32, tag=f"o{2 * b + j}",
                               name=f"sv{b}{j}") for j in range(2)]
                ops_b[b] = ops
                wts = {}
                for t in range(NDR + PIPE):
                    if t < NDR:
                        wt = wtpool.tile([P, 2, P], fp8, tag="wt", name="wt")
                        for i in range(2):
                            tpw = pp.tile([P, P], f32, tag=f"t{(2 * t + i) % 2}",
                                          name="tpw")
                            nc.tensor.matmul(
                                tpw[:],
                                W_s[b][:, (2 * t + i) * P:(2 * t + i + 1) * P],
                                ident_8[:], start=True, stop=True,
                            )
                            nc.vector.tensor_copy(out=wt[:, i, :], in_=tpw[:])
                        wts[t] = wt
                    if extra and t < len(extra):
                        extra[t]()
                    if t >= PIPE:
                        t_ = t - PIPE
                        vt = vts[(b, t_ // 4)]
                        tt = t_ % 4
                        for j in range(2):
                            nc.tensor.matmul(
                                ops[j][:], wts[t_][:],
                                vt[:, 2 * tt:2 * tt + 2, j * 512:(j + 1) * 512],
                                start=(t_ == 0), stop=False, perf_mode=DR,
                            )
                        del wts[t_]

            def scores_newkey(b):
                # scores against the T new keys + normalization + padded
                # transposed new-key weights (last S@V chunk lhsT)
                spn = pp.tile([P, 512], f32, tag=f"s{b % 2}", name="spn")
                for ci in range(0, CH, 2):
                    nc.tensor.matmul(
                        spn[:, 0:T], Qb[b][:, ci:ci + 2, :],
                        kT[:, ci:ci + 2, b * T:(b + 1) * T],
                        start=(ci == 0), stop=(ci == CH - 2), perf_mode=DR,
                    )
                nc.scalar.activation(
                    W_s[b][:, L:LT], spn[:, 0:T], AF.Exp,
                    bias=eshift[:, 0:1], accum_out=sums[b][:, NW:NW + 1],
                )
                rs = big.tile([P, 1], f32, tag=f"rs{b}", name=f"rs{b}")
                nc.vector.tensor_reduce(out=rs[:], in_=sums[b][:],
                                        axis=AX.X, op=OP.add)
                rsum[b] = big.tile([P, 1], f32, tag=f"rsum{b}", name=f"rsum{b}")
                nc.vector.reciprocal(rsum[b][:], rs[:])
                wn_pad = big.tile([P, M], fp8, tag=f"wn_pad{b}",
                                  name=f"wn_pad{b}")
                nc.gpsimd.memset(wn_pad[:], 0.0)
                nc.vector.tensor_copy(out=wn_pad[:, b * T:(b + 1) * T],
                                      in_=W_s[b][:, L:LT])
                tpn = pp.tile([M, P], f32, tag=f"t{b % 2}", name="tpn")
                nc.tensor.matmul(tpn[:], wn_pad[:], ident_8[:],
                                 start=True, stop=True)
                wt32[b] = big.tile([P, P], fp8, tag=f"wt32_{b}",
                                   name=f"wt32_{b}")
                nc.gpsimd.memset(wt32[b][:], 0.0)
                nc.vector.tensor_copy(out=wt32[b][0:M, :], in_=tpn[:])

            def sv_final(b):
                ops = ops_b[b]
                for j in range(2):
                    nc.tensor.matmul(
                        ops[j][:], wt32[b][:], vpad[:, j * 512:(j + 1) * 512],
                        start=False, stop=True,
                    )
                for j in range(2):
                    nc.scalar.activation(
                        On[b][:, j * 512:(j + 1) * 512], ops[j][:], AF.Copy,
                        scale=rsum[b][:],
                    )

            # wvT[c_local, (pair), (h,t)]: head of c_local<64 is 2ci else
            # 2ci+1 -> two 64-partition block copies per (b, ci).
            wvT = big.tile([P, CH, M], fp8, tag="wvT", name="wvT")

            def gather_ci(b, ci):
                tp = pp.tile([P, P], f32, tag=f"t{ci % 2}", name="tpg")
                nc.tensor.matmul(tp[:], On[b][:, ci * P:(ci + 1) * P],
                                 ident_b[:], start=True, stop=True)
                nc.vector.tensor_copy(
                    out=wvT[0:64, ci, b * T:(b + 1) * T],
                    in_=tp[0:64, 16 * ci:16 * ci + 8])
                nc.vector.tensor_copy(
                    out=wvT[64:P, ci, b * T:(b + 1) * T],
                    in_=tp[64:P, 16 * ci + 8:16 * ci + 16])

            # ---------------- Phase B/C: batch 0 scores + S@V --------------
            scores_windows(0)
            sv_main(0)

            # ---------------- Phase D: batch 1 scores ----------------------
            scores_windows(1)

            # ---------------- Phase E: k/v natural projections (bf16) ------
            k_nat = natpool.tile([M, C], f32, tag="nat", name="k_nat")
            v_nat = natpool.tile([M, C], f32, tag="nat", name="v_nat")
            k_bf = big.tile([M, C], bf16, tag="k_bf", name="k_bf")
            for half, nat in ((0, k_nat), (1, v_nat)):
                for j in range(2):
                    ps = pp.tile([M, 512], f32, tag=f"s{j}", name=f"kv{half}{j}")
                    for ci in range(CH):
                        nc.tensor.matmul(
                            ps[:], xTb_sb[:, ci, :],
                            wkv_sb[:, ci, half * C + j * 512:
                                   half * C + (j + 1) * 512],
                            start=(ci == 0), stop=(ci == CH - 1),
                        )
                    sl = slice(j * 512, (j + 1) * 512)
                    if half == 0:
                        nc.scalar.copy(nat[:, sl], ps[:])
                        nc.vector.tensor_copy(out=k_bf[:, sl], in_=ps[:])
                    else:
                        nc.vector.tensor_add(out=nat[:, sl], in0=ps[:],
                                             in1=bvb_sb[:, sl])
            nc.scalar.dma_start(key_d[:], k_nat[:])
            nc.scalar.dma_start(val_d[:], v_nat[:])
            # kT (new keys, fp8) via matmul-transpose of k_bf chunks
            kT = big.tile([P, CH, M], fp8, tag="kT", name="kT")
            for ci in range(CH):
                tp = pp.tile([P, M], f32, tag=f"t{ci % 2}", name="tpk")
                nc.tensor.matmul(tp[:], k_bf[:, ci * P:(ci + 1) * P],
                                 ident_b[0:M, 0:M], start=True, stop=True)
                nc.vector.tensor_copy(out=kT[:, ci, :], in_=tp[:])
            # padded new-v rows for the last S@V chunk (both batches)
            vpad = big.tile([P, C], fp8, tag="vpad", name="vpad")
            nc.gpsimd.memset(vpad[:], 0.0)
            nc.vector.tensor_copy(out=vpad[0:M, :], in_=v_nat[:])

            # ---------------- Phase F: batch 0 finish -----------------------
            scores_newkey(0)
            sv_final(0)

            # ---------------- Phase G: batch 1 S@V + gather(0) --------------
            sv_main(1, extra=[lambda ci=ci: gather_ci(0, ci)
                              for ci in range(CH)])

            # ---------------- Phase H: batch 1 finish -----------------------
            scores_newkey(1)
            sv_final(1)

            # ---------------- Phase I: gather(1) + out projection (DR) ------
            ps_fin = [pp.tile([M, 512], f32, tag=f"s{j}", name=f"fin{j}")
                      for j in range(2)]
            for ci in range(CH):
                gather_ci(1, ci)
                if ci % 2 == 1:
                    for j in range(2):
                        nc.tensor.matmul(
                            ps_fin[j][:], wvT[:, ci - 1:ci + 1, :],
                            wo_sb[:, ci - 1:ci + 1, j * 512:(j + 1) * 512],
                            start=(ci == 1), stop=(ci == CH - 1), perf_mode=DR,
                        )
            fin = natpool.tile([M, C], f32, tag="nat", name="fin")
            for j in range(2):
                sl = slice(j * 512, (j + 1) * 512)
                nc.vector.tensor_add(out=fin[:, sl], in0=ps_fin[j][:],
                                     in1=bob_sb[:, sl])
                nc.scalar.dma_start(out_d[:, sl], fin[:, sl])

    nc.compile()
    return nc


def _prep_host(x, kv_cache, Wq, bq, Wk, Wv, bv, Wo, bo):
    fp8 = ml_dtypes.float8_e4m3
    bf16 = ml_dtypes.bfloat16
    f32 = np.float32
    x = np.asarray(x, f32)
    kv = np.asarray(kv_cache)
    Wq = np.asarray(Wq, f32); bq = np.asarray(bq, f32)
    Wk = np.asarray(Wk, f32); Wv = np.asarray(Wv, f32); bv = np.asarray(bv, f32)
    Wo = np.asarray(Wo, f32); bo = np.asarray(bo, f32)

    # K-cache / V-cache repacked so every device DMA is a fully contiguous
    # [128 x >=4KB] transfer:
    #   KT[b, w, p, ci*512 + j] = K[b, w*512 + j, ci*128 + p]
    #   Vd[b, s, p, tt*C + c]   = V[b, (s*NV + tt)*128 + p, c]
    KT_all = np.asarray(kv[:, 1, 0], f32).transpose(0, 2, 1).reshape(
        B, CH, P, NW, 512).transpose(0, 3, 2, 1, 4)
    KT_all = np.ascontiguousarray(KT_all).astype(fp8)
    V_all = np.asarray(kv[:, 1, 1], f32).reshape(
        B, L // (P * NV), NV, P, C).transpose(0, 1, 3, 2, 4)
    V_all = np.ascontiguousarray(V_all).astype(fp8)

    # weights: [P, CH, C(out)] with c_in = ci*128 + p
    Wq8 = np.ascontiguousarray(
        (Wq.T * SCALE).reshape(CH, P, C).transpose(1, 0, 2)).astype(fp8)
    Wo8 = np.ascontiguousarray(
        Wo.T.reshape(CH, P, C).transpose(1, 0, 2)).astype(fp8)
    Wkv8 = np.ascontiguousarray(
        np.concatenate([Wk.T, Wv.T], axis=1).reshape(CH, P, 2 * C)
        .transpose(1, 0, 2)).astype(bf16)
    bqs = np.ascontiguousarray((bq * SCALE).reshape(CH, P).T)  # [P, CH]
    bvb = np.ascontiguousarray(np.tile(bv, (M, 1)))
    bob = np.ascontiguousarray(np.tile(bo, (M, 1)))

    in_maps = []
    for c in range(NCORES):
        xc = x[c * BPC:(c + 1) * BPC].reshape(M, C)
        xT = np.ascontiguousarray(xc.reshape(M, CH, P).transpose(2, 1, 0))
        in_maps.append({
            "xT8": xT.astype(fp8),
            "xTb": xT.astype(bf16),
            "Wq8": Wq8.reshape(P, CH * C),
            "Wo8": Wo8.reshape(P, CH * C),
            "Wkvd": Wkv8.reshape(P, CH * 2 * C),
            "KT": np.ascontiguousarray(KT_all[c * BPC:(c + 1) * BPC]).reshape(
                BPC, NW, P, CH * 512),
            "Vd": np.ascontiguousarray(V_all[c * BPC:(c + 1) * BPC]).reshape(
                BPC, L // (P * NV), P, NV * C),
            "bqs": bqs, "bvb": bvb, "bob": bob,
        })
    return in_maps


def kernel(x, kv_cache, Wq, bq, Wk, Wv, bv, Wo, bo, _trace=False, _tmpdir=None):
    from concourse.bass_utils import run_bass_kernel_spmd

    _ensure_ntff_hook()
    if "nc" not in _CACHE:
        _CACHE["nc"] = _build()
    nc = _CACHE["nc"]

    in_maps = _prep_host(x, kv_cache, Wq, bq, Wk, Wv, bv, Wo, bo)
    res = run_bass_kernel_spmd(
        nc, in_maps, core_ids=list(range(NCORES)),
        trace=_trace, tmpdir=_tmpdir,
    )
    out = np.empty((B, T, C), np.float32)
    key_o = np.empty((B, T, C), np.float32)
    val_o = np.empty((B, T, C), np.float32)
    for c in range(NCORES):
        r = res.results[c]
        sl = slice(c * BPC, (c + 1) * BPC)
        out[sl] = r["out"].reshape(BPC, T, C)
        key_o[sl] = r["key"].reshape(BPC, T, C)
        val_o[sl] = r["value"].reshape(BPC, T, C)
    kernel._last_exec_time_ns = res.exec_time_ns
    kernel._last_results = res
    return (out, key_o, val_o)
